# revision 8
# baseline (speedup 1.0000x reference)
"""Multi-head attention (B=2, S=2048, nx=768, H=12) on 8 TRN2 NeuronCores.

Sharding: 24 (batch, head) pairs -> 3 heads per core. Core c handles batch
c//4, heads {3*(c%4), +1, +2}. Each core computes QKV projection for its
head slice, attention, and a partial output projection (its 192 rows of
w_proj); the host sums the 4 partials per batch and adds b_proj.

Device layout (per core, all matmul operands bf16, accumulation f32):
  - Inputs arrive pre-transposed/sliced from host:
      xt  [896, 2048]  = [hidden[b].T ; ones-row ; zero-pad]  (bias trick)
      wqk [896, 384]   = w_attn cols [qA kA qB kB qC kC] + bias row
      wv  [896, 192]   = w_attn v cols [vA vB vC] + bias row
      wp  [192, 768]   = w_proj rows for the 3 heads
  - QK proj emits Q^T/K^T directly ([d, s] layout) so the transposed-score
    matmul S^T[t, q] = K Q^T needs no transposes anywhere.
  - softmax: scores stay small (no max subtraction needed); exp on ACT with
    fused 1/8 scale; denominator via a ones-column appended to V (the PV
    matmul then emits sum(exp) as row 64); normalize with reciprocal +
    K=1-matmul partition broadcast.
  - output proj consumes a^T [192, s] as lhsT, wp as rhs -> natural [s, n]
    partial written straight from PSUM to DRAM.
"""

import numpy as np
import ml_dtypes

import concourse.bass as bass
import concourse.tile as tile
import concourse.mybir as mybir
from concourse import bacc

BF16 = mybir.dt.bfloat16
F32 = mybir.dt.float32

NX = 768
D = 64
HPC = 3          # heads per core
N_CORES = 8
KCH = 7          # contraction chunks of 128 (768 data + bias row + pad)
KDIM = KCH * 128  # 896


def build_nc(S=2048):
    """Build the single-core SPMD program. S = sequence length."""
    TC = S // 128    # t (key) chunks
    QC = S // 512    # q chunks of 512
    nc = bacc.Bacc("TRN2", target_bir_lowering=False, debug=False)

    xt_d = nc.dram_tensor("xt", [KDIM, S], BF16, kind="ExternalInput")
    wqk_d = nc.dram_tensor("wqk", [KDIM, 6 * D], BF16, kind="ExternalInput")
    wv_d = nc.dram_tensor("wv", [KDIM, HPC * D], BF16, kind="ExternalInput")
    wp_d = nc.dram_tensor("wp", [HPC * D, NX], BF16, kind="ExternalInput")
    out_d = nc.dram_tensor("out", [S, NX], F32, kind="ExternalOutput")

    with tile.TileContext(nc) as tc:
        _build_body(tc, out_d.ap(), xt_d.ap(), wqk_d.ap(), wv_d.ap(),
                    wp_d.ap(), S, TC, QC)
    nc.compile()
    return nc


def _build_body(tc, out_d, xt_d, wqk_d, wv_d, wp_d, S, TC, QC):
    nc = tc.nc
    P = 128

    with tc.tile_pool(name="const", bufs=1) as cpool, \
         tc.tile_pool(name="work", bufs=1) as wpool, \
         tc.tile_pool(name="small", bufs=3) as spool, \
         tc.tile_pool(name="ps_small", bufs=3, space="PSUM") as ps_small, \
         tc.tile_pool(name="ps_score", bufs=2, space="PSUM") as ps_score, \
         tc.tile_pool(name="ps_pv", bufs=1, space="PSUM") as ps_pv:

        # ---- stage inputs in SBUF ----
        xt_sb = cpool.tile([P, KCH, S], BF16)
        nc.sync.dma_start(xt_sb[:], xt_d.rearrange("(c p) s -> p c s", p=P))
        wqk_sb = cpool.tile([P, KCH, 6 * D], BF16)
        nc.sync.dma_start(wqk_sb[:], wqk_d.rearrange("(c p) m -> p c m", p=P))
        wv_sb = cpool.tile([P, KCH, HPC * D], BF16)
        nc.sync.dma_start(wv_sb[:], wv_d.rearrange("(c p) m -> p c m", p=P))
        wp0_sb = cpool.tile([P, NX], BF16)
        nc.sync.dma_start(wp0_sb[:], wp_d[0:P, :])
        wp1_sb = cpool.tile([D, NX], BF16)
        nc.sync.dma_start(wp1_sb[:], wp_d[P:HPC * D, :])
        ones_sb = cpool.tile([1, D], F32)
        nc.vector.memset(ones_sb[:], 1.0)

        # ---- QK projection -> per-head Q^T/K^T strips at base partition 0
        # wqk col order is [qA kA qB kB qC kC]; m-chunk mc covers head mc's
        # q (partitions 0:64) and k (64:128).
        q_sb = cpool.tile([D, HPC, S], BF16)
        k_sb = cpool.tile([D, HPC, S], BF16)
        for mc in range(HPC):
            for qc in range(QC):
                ps = ps_small.tile([P, 512], F32, tag="ps_small")
                for kc in range(KCH):
                    nc.tensor.matmul(
                        ps[:],
                        wqk_sb[:, kc, mc * 128:(mc + 1) * 128],
                        xt_sb[:, kc, qc * 512:(qc + 1) * 512],
                        start=(kc == 0), stop=(kc == KCH - 1))
                nc.vector.tensor_copy(q_sb[:, mc, qc * 512:(qc + 1) * 512],
                                      ps[0:D, :])
                nc.vector.tensor_copy(k_sb[:, mc, qc * 512:(qc + 1) * 512],
                                      ps[D:P, :])

        # ---- V projection -> v_sb [tok_part, tc, head, 65] (col 64 = ones)
        v_sb = cpool.tile([P, TC, HPC, D + 1], BF16)
        nc.vector.memset(v_sb[:, :, :, D:D + 1], 1.0)
        for t in range(TC):
            ps = ps_small.tile([P, 512], F32, tag="ps_small")
            for kc in range(KCH):
                nc.tensor.matmul(
                    ps[:, 0:HPC * D],
                    xt_sb[:, kc, t * 128:(t + 1) * 128],
                    wv_sb[:, kc, :],
                    start=(kc == 0), stop=(kc == KCH - 1))
            nc.vector.tensor_copy(
                v_sb[:, t, :, 0:D],
                ps[:, 0:HPC * D].rearrange("p (h d) -> p h d", h=HPC))

        # ---- attention per head ----
        aT_ab = wpool.tile([P, S], BF16, tag="aT_ab")   # heads 0,1 stacked
        aT_c = wpool.tile([D, S], BF16, tag="aT_c")     # head 2
        for h in range(HPC):
            # scores^T + exp: E[t, q] = exp((K Q^T)[t, q] / 8)
            e_sb = wpool.tile([P, TC, S], BF16, tag="E")
            for t in range(TC):
                for half in range(S // 1024):
                    sps = ps_score.tile([P, 1024], F32, tag="score")
                    for qq in range(2):
                        qc = half * 2 + qq
                        nc.tensor.matmul(
                            sps[:, qq * 512:(qq + 1) * 512],
                            k_sb[:, h, t * 128:(t + 1) * 128],
                            q_sb[:, h, qc * 512:(qc + 1) * 512],
                            start=True, stop=True)
                    nc.scalar.activation(
                        e_sb[:, t, half * 1024:(half + 1) * 1024], sps[:],
                        mybir.ActivationFunctionType.Exp, scale=0.125)
            # PV with ones column: psum rows 0:64 = a^T unnormalized,
            # row 64 = sum(exp)
            for qc in range(QC):
                pv = ps_pv.tile([P, 512], F32, tag="pv")
                for t in range(TC):
                    nc.tensor.matmul(
                        pv[0:D + 1, :],
                        v_sb[:, t, h, :],
                        e_sb[:, t, qc * 512:(qc + 1) * 512],
                        start=(t == 0), stop=(t == TC - 1))
                r = spool.tile([1, 512], F32, tag="recip")
                nc.vector.reciprocal(r[:], pv[D:D + 1, :])
                rb = ps_small.tile([P, 512], F32, tag="ps_small")
                nc.tensor.matmul(rb[0:D, :], ones_sb[:], r[:],
                                 start=True, stop=True)
                rb_sb = spool.tile([D, 512], F32, tag="rb_sb")
                nc.vector.tensor_copy(rb_sb[:], rb[0:D, :])
                dst = (aT_ab[h * D:(h + 1) * D, qc * 512:(qc + 1) * 512]
                       if h < 2 else aT_c[:, qc * 512:(qc + 1) * 512])
                nc.vector.tensor_tensor(dst, pv[0:D, :], rb_sb[:],
                                        mybir.AluOpType.mult)

        # ---- output projection: out[s, n] partial, straight PSUM -> DRAM
        for sc in range(S // 128):
            s_sl = slice(sc * 128, (sc + 1) * 128)
            for n0, nw in ((0, 512), (512, 256)):
                ps = ps_small.tile([P, 512], F32, tag="ps_small")
                nc.tensor.matmul(ps[:, 0:nw], aT_ab[:, s_sl],
                                 wp0_sb[:, n0:n0 + nw],
                                 start=True, stop=False)
                nc.tensor.matmul(ps[:, 0:nw], aT_c[:, s_sl],
                                 wp1_sb[:, n0:n0 + nw],
                                 start=False, stop=True)
                ostage = spool.tile([P, 512], F32, tag="ostage")
                nc.vector.tensor_copy(ostage[:, 0:nw], ps[:, 0:nw])
                nc.sync.dma_start(out_d[s_sl, n0:n0 + nw], ostage[:, 0:nw])


# ---------------------------------------------------------------------------
# host side
# ---------------------------------------------------------------------------

def make_in_maps(hidden_states, w_attn, b_attn, w_proj, S=2048):
    """Build the 8 per-core input dicts (numpy bf16)."""
    bf = ml_dtypes.bfloat16
    hidden = np.asarray(hidden_states)
    w_attn = np.asarray(w_attn)
    b_attn = np.asarray(b_attn)
    w_proj = np.asarray(w_proj)

    xts = []
    for b in range(hidden.shape[0]):
        xt = np.zeros((KDIM, S), dtype=bf)
        xt[0:NX, :] = hidden[b].T.astype(bf)
        xt[NX, :] = 1.0
        xts.append(xt)

    in_maps = []
    for c in range(N_CORES):
        b = c // (N_CORES // hidden.shape[0])
        h0 = HPC * (c % (N_CORES // hidden.shape[0]))
        wqk = np.zeros((KDIM, 6 * D), dtype=bf)
        wv = np.zeros((KDIM, HPC * D), dtype=bf)
        for i in range(HPC):
            h = h0 + i
            wqk[0:NX, (2 * i) * D:(2 * i + 1) * D] = \
                w_attn[:, h * D:(h + 1) * D].astype(bf)
            wqk[NX, (2 * i) * D:(2 * i + 1) * D] = \
                b_attn[h * D:(h + 1) * D].astype(bf)
            wqk[0:NX, (2 * i + 1) * D:(2 * i + 2) * D] = \
                w_attn[:, NX + h * D:NX + (h + 1) * D].astype(bf)
            wqk[NX, (2 * i + 1) * D:(2 * i + 2) * D] = \
                b_attn[NX + h * D:NX + (h + 1) * D].astype(bf)
            wv[0:NX, i * D:(i + 1) * D] = \
                w_attn[:, 2 * NX + h * D:2 * NX + (h + 1) * D].astype(bf)
            wv[NX, i * D:(i + 1) * D] = \
                b_attn[2 * NX + h * D:2 * NX + (h + 1) * D].astype(bf)
        wp = w_proj[h0 * D:(h0 + HPC) * D, :].astype(bf)
        in_maps.append({"xt": xts[b], "wqk": wqk, "wv": wv, "wp": wp})
    return in_maps


_CACHE = {}


def kernel(hidden_states, w_attn, b_attn, w_proj, b_proj):
    from concourse.bass_utils import run_bass_kernel_spmd

    hidden = np.asarray(hidden_states, dtype=np.float32)
    B, S, _ = hidden.shape
    in_maps = make_in_maps(hidden, w_attn, b_attn, w_proj, S=S)

    if S not in _CACHE:
        _CACHE[S] = build_nc(S=S)
    nc = _CACHE[S]

    res = run_bass_kernel_spmd(nc, in_maps, core_ids=list(range(N_CORES)))
    cpb = N_CORES // B
    out = np.zeros((B, S, NX), dtype=np.float32)
    for c in range(N_CORES):
        out[c // cpb] += res.results[c]["out"]
    out += np.asarray(b_proj, dtype=np.float32)
    return out


# revision 13
# speedup vs baseline: 1.2699x; 1.2699x over previous
"""Multi-head attention (B=2, S=2048, nx=768, H=12) on 8 TRN2 NeuronCores.

Sharding: 24 (batch, head) pairs -> 3 heads per core. Core c handles batch
c//4, heads {3*(c%4), +1, +2}. Each core computes QKV projection for its
head slice, attention, and a partial output projection (its 192 rows of
w_proj); the host sums the 4 partials per batch and adds b_proj.

Device pipeline (per core, matmul operands bf16, accumulation f32):
  - Inputs arrive pre-transposed/sliced from host:
      xt  [896, 2048]  = [hidden[b].T ; ones-row ; zero-pad]  (bias trick)
      wqk [896, 384]   = w_attn cols [qA kA qB kB qC kC] + bias row
      wv  [896, 192]   = w_attn v cols [vA vB vC] + bias row
      wp  [192, 768]   = w_proj rows for the 3 heads
  - QK proj emits Q^T/K^T directly ([d, s] layout) so the transposed-score
    matmul S^T[t, q] = K Q^T needs no transposes anywhere.
  - softmax: scores are small (no max subtraction needed); exp on ACT with
    fused 1/8 scale; denominator via a ones-column appended to V (the PV
    matmul emits sum(exp) as psum row 64); normalize with reciprocal +
    K=1-matmul partition broadcast.
  - Head pipeline: PV for head h runs t-major (4 open psum accumulations,
    one per 512-query chunk) interleaved with scores+exp for head h+1, so
    the scalar engine (exp is its ~110us floor) never starves and the PE
    never idles long enough to re-throttle (HAM).
  - output proj consumes a^T [192, s] as lhsT, wp as rhs -> natural [s, n]
    partial, staged through SBUF to DRAM.
"""

import numpy as np
import ml_dtypes

import concourse.bass as bass
import concourse.tile as tile
import concourse.mybir as mybir
from concourse import bacc

BF16 = mybir.dt.bfloat16
F32 = mybir.dt.float32

NX = 768
D = 64
HPC = 3          # heads per core
N_CORES = 8
KCH = 7          # contraction chunks of 128 (768 data + bias row + pad)
KDIM = KCH * 128  # 896


def build_nc(S=2048):
    """Build the single-core SPMD program. S = sequence length."""
    TC = S // 128    # t (key) chunks
    QC = S // 512    # q chunks of 512
    nc = bacc.Bacc("TRN2", target_bir_lowering=False, debug=False)

    xt_d = nc.dram_tensor("xt", [KDIM, S], BF16, kind="ExternalInput")
    wqk_d = nc.dram_tensor("wqk", [KDIM, 6 * D], BF16, kind="ExternalInput")
    wv_d = nc.dram_tensor("wv", [KDIM, HPC * D], BF16, kind="ExternalInput")
    wp_d = nc.dram_tensor("wp", [HPC * D, NX], BF16, kind="ExternalInput")
    out_d = nc.dram_tensor("out", [S, NX], F32, kind="ExternalOutput")

    with tile.TileContext(nc) as tc:
        _build_body(tc, out_d.ap(), xt_d.ap(), wqk_d.ap(), wv_d.ap(),
                    wp_d.ap(), S, TC, QC)
    nc.compile()
    return nc


def _build_body(tc, out_d, xt_d, wqk_d, wv_d, wp_d, S, TC, QC):
    nc = tc.nc
    P = 128
    NHALF = S // 1024  # exp calls per t-chunk, each [128, 1024]

    with tc.tile_pool(name="const", bufs=1) as cpool, \
         tc.tile_pool(name="epool", bufs=TC + 2) as epool, \
         tc.tile_pool(name="small", bufs=3) as spool, \
         tc.tile_pool(name="ps_score", bufs=2, space="PSUM") as ps_score, \
         tc.tile_pool(name="ps_pv", bufs=QC, space="PSUM") as ps_pv:

        # ---- stage inputs in SBUF ----
        xt_sb = cpool.tile([P, KCH, S], BF16)
        nc.sync.dma_start(xt_sb[:], xt_d.rearrange("(c p) s -> p c s", p=P))
        wqk_sb = cpool.tile([P, KCH, 6 * D], BF16)
        nc.sync.dma_start(wqk_sb[:], wqk_d.rearrange("(c p) m -> p c m", p=P))
        wv_sb = cpool.tile([P, KCH, HPC * D], BF16)
        nc.sync.dma_start(wv_sb[:], wv_d.rearrange("(c p) m -> p c m", p=P))
        wp0_sb = cpool.tile([P, NX], BF16)
        nc.sync.dma_start(wp0_sb[:], wp_d[0:P, :])
        wp1_sb = cpool.tile([D, NX], BF16)
        nc.sync.dma_start(wp1_sb[:], wp_d[P:HPC * D, :])
        ones_sb = cpool.tile([1, D], F32)
        nc.vector.memset(ones_sb[:], 1.0)

        q_sb = cpool.tile([D, HPC, S], BF16)
        k_sb = cpool.tile([D, HPC, S], BF16)
        v_sb = cpool.tile([P, TC, HPC, D + 1], BF16)
        aT_ab = cpool.tile([P, S], BF16)   # heads 0,1 stacked
        aT_c = cpool.tile([D, S], BF16)    # head 2

        # wqk col order is [qA kA qB kB qC kC]; m-chunk mc covers head mc's
        # q (psum partitions 0:64) and k (64:128).
        def qk_proj(mc):
            for qc in range(QC):
                ps = ps_score.tile([P, 1024], F32, tag="score")
                for kc in range(KCH):
                    nc.tensor.matmul(
                        ps[:, 0:512],
                        wqk_sb[:, kc, mc * 128:(mc + 1) * 128],
                        xt_sb[:, kc, qc * 512:(qc + 1) * 512],
                        start=(kc == 0), stop=(kc == KCH - 1))
                nc.vector.tensor_copy(q_sb[:, mc, qc * 512:(qc + 1) * 512],
                                      ps[0:D, 0:512])
                nc.vector.tensor_copy(k_sb[:, mc, qc * 512:(qc + 1) * 512],
                                      ps[D:P, 0:512])

        def v_proj():
            nc.vector.memset(v_sb[:, :, :, D:D + 1], 1.0)
            for t in range(TC):
                ps = ps_score.tile([P, 1024], F32, tag="score")
                for kc in range(KCH):
                    nc.tensor.matmul(
                        ps[:, 0:HPC * D],
                        xt_sb[:, kc, t * 128:(t + 1) * 128],
                        wv_sb[:, kc, :],
                        start=(kc == 0), stop=(kc == KCH - 1))
                nc.vector.tensor_copy(
                    v_sb[:, t, :, 0:D],
                    ps[:, 0:HPC * D].rearrange("p (h d) -> p h d", h=HPC))

        e_tiles = {}

        def scores_exp(h, t):
            e = epool.tile([P, S], BF16, tag="E")
            e_tiles[(h, t)] = e
            for half in range(NHALF):
                sps = ps_score.tile([P, 1024], F32, tag="score")
                for qq in range(2):
                    qc = half * 2 + qq
                    nc.tensor.matmul(
                        sps[:, qq * 512:(qq + 1) * 512],
                        k_sb[:, h, t * 128:(t + 1) * 128],
                        q_sb[:, h, qc * 512:(qc + 1) * 512],
                        start=True, stop=True)
                nc.scalar.activation(
                    e[:, half * 1024:(half + 1) * 1024], sps[:],
                    mybir.ActivationFunctionType.Exp, scale=0.125)

        def norm(h, qc, pv):
            r = spool.tile([1, 512], F32, tag="recip")
            nc.vector.reciprocal(r[:], pv[D:D + 1, :])
            rb = ps_score.tile([P, 1024], F32, tag="score")
            nc.tensor.matmul(rb[0:D, 0:512], ones_sb[:], r[:],
                             start=True, stop=True)
            rb_sb = spool.tile([D, 512], F32, tag="rb_sb")
            nc.vector.tensor_copy(rb_sb[:], rb[0:D, 0:512])
            dst = (aT_ab[h * D:(h + 1) * D, qc * 512:(qc + 1) * 512]
                   if h < 2 else aT_c[:, qc * 512:(qc + 1) * 512])
            nc.vector.tensor_tensor(dst, pv[0:D, :], rb_sb[:],
                                    mybir.AluOpType.mult)

        # ---- emission order = pipeline order ----
        qk_proj(0)
        for t in range(TC):
            scores_exp(0, t)
        qk_proj(1)
        qk_proj(2)
        v_proj()

        for h in range(HPC):
            pvs = [ps_pv.tile([P, 512], F32, tag="pv", name=f"pv_{h}_{qc}")
                   for qc in range(QC)]
            for t in range(TC):
                if h + 1 < HPC:
                    scores_exp(h + 1, t)
                e = e_tiles.pop((h, t))
                for qc in range(QC):
                    nc.tensor.matmul(
                        pvs[qc][0:D + 1, :],
                        v_sb[:, t, h, :],
                        e[:, qc * 512:(qc + 1) * 512],
                        start=(t == 0), stop=(t == TC - 1))
            for qc in range(QC):
                norm(h, qc, pvs[qc])

        # ---- output projection: out[s, n] partial -> DRAM ----
        for sc in range(S // 128):
            s_sl = slice(sc * 128, (sc + 1) * 128)
            for n0, nw in ((0, 512), (512, 256)):
                ps = ps_score.tile([P, 1024], F32, tag="score")
                nc.tensor.matmul(ps[:, 0:nw], aT_ab[:, s_sl],
                                 wp0_sb[:, n0:n0 + nw],
                                 start=True, stop=False)
                nc.tensor.matmul(ps[:, 0:nw], aT_c[:, s_sl],
                                 wp1_sb[:, n0:n0 + nw],
                                 start=False, stop=True)
                ostage = spool.tile([P, 512], F32, tag="ostage")
                nc.vector.tensor_copy(ostage[:, 0:nw], ps[:, 0:nw])
                nc.sync.dma_start(out_d[s_sl, n0:n0 + nw], ostage[:, 0:nw])


# ---------------------------------------------------------------------------
# host side
# ---------------------------------------------------------------------------

def make_in_maps(hidden_states, w_attn, b_attn, w_proj, S=2048):
    """Build the 8 per-core input dicts (numpy bf16)."""
    bf = ml_dtypes.bfloat16
    hidden = np.asarray(hidden_states)
    w_attn = np.asarray(w_attn)
    b_attn = np.asarray(b_attn)
    w_proj = np.asarray(w_proj)

    xts = []
    for b in range(hidden.shape[0]):
        xt = np.zeros((KDIM, S), dtype=bf)
        xt[0:NX, :] = hidden[b].T.astype(bf)
        xt[NX, :] = 1.0
        xts.append(xt)

    in_maps = []
    for c in range(N_CORES):
        b = c // (N_CORES // hidden.shape[0])
        h0 = HPC * (c % (N_CORES // hidden.shape[0]))
        wqk = np.zeros((KDIM, 6 * D), dtype=bf)
        wv = np.zeros((KDIM, HPC * D), dtype=bf)
        for i in range(HPC):
            h = h0 + i
            wqk[0:NX, (2 * i) * D:(2 * i + 1) * D] = \
                w_attn[:, h * D:(h + 1) * D].astype(bf)
            wqk[NX, (2 * i) * D:(2 * i + 1) * D] = \
                b_attn[h * D:(h + 1) * D].astype(bf)
            wqk[0:NX, (2 * i + 1) * D:(2 * i + 2) * D] = \
                w_attn[:, NX + h * D:NX + (h + 1) * D].astype(bf)
            wqk[NX, (2 * i + 1) * D:(2 * i + 2) * D] = \
                b_attn[NX + h * D:NX + (h + 1) * D].astype(bf)
            wv[0:NX, i * D:(i + 1) * D] = \
                w_attn[:, 2 * NX + h * D:2 * NX + (h + 1) * D].astype(bf)
            wv[NX, i * D:(i + 1) * D] = \
                b_attn[2 * NX + h * D:2 * NX + (h + 1) * D].astype(bf)
        wp = w_proj[h0 * D:(h0 + HPC) * D, :].astype(bf)
        in_maps.append({"xt": xts[b], "wqk": wqk, "wv": wv, "wp": wp})
    return in_maps


_CACHE = {}


def kernel(hidden_states, w_attn, b_attn, w_proj, b_proj):
    from concourse.bass_utils import run_bass_kernel_spmd

    hidden = np.asarray(hidden_states, dtype=np.float32)
    B, S, _ = hidden.shape
    in_maps = make_in_maps(hidden, w_attn, b_attn, w_proj, S=S)

    if S not in _CACHE:
        _CACHE[S] = build_nc(S=S)
    nc = _CACHE[S]

    res = run_bass_kernel_spmd(nc, in_maps, core_ids=list(range(N_CORES)))
    cpb = N_CORES // B
    out = np.zeros((B, S, NX), dtype=np.float32)
    for c in range(N_CORES):
        out[c // cpb] += res.results[c]["out"]
    out += np.asarray(b_proj, dtype=np.float32)
    return out
